# revision 14
# baseline (speedup 1.0000x reference)
"""Trainium2 Bass kernel for nn_AlignModel (40-view / 4-marker projective
alignment forward pass).  HW-verified: rel err 2.04e-07 vs the jax
reference.

Algorithm (mirrors the reference):
  tilt_deg = W @ tilt_angles + c (static interpolation weights, view 14
  pinned to -15 deg) -> trig via ScalarE Sin (cos x = sin(pi/2 - |x|))
  -> one [1,80] VectorE multiply forms mag*cos|mag*sin (mag doubled in
  the blob), six interleaved products build the per-view projection rows
  -> uv = X' @ [off; A] via four K=1 accumulating rank-1 matmuls on PE
  (PSUM accumulation does the cross-partition fan-out; the offset row is
  read straight from the pack tile) -> one DMA out; the host drops the 18
  statically-skipped rows.  View-0 mag/offset overrides are constant
  injections in the host-packed blob, like the ones rows.

Perf notes (neuron-profile on trn2; ~21 device instructions):
  - ONE blob input DMA: input DMAs serialize on the SP sequencer
    (~800ns each) and each costs ~1us completion latency.
  - This walrus accepts ONE sem-wait per instruction; per-engine order is
    pinned (sync=False dep edges) so every instruction has at most one
    unobserved producer; a post-pass hoists leftovers onto single-wait
    EventSemaphore carriers.
  - NEFF preamble (const-AP memsets + init barrier) stripped; nothing
    uses the const APs.  The tail barrier must stay (NRT completion).
  - Replicated on all 8 NeuronCores (too small to shard); core 0's
    output is returned."""

import numpy as np

import concourse.bass as bass
import concourse.mybir as mybir
from concourse import tile
from concourse.tile_rust import add_dep_helper
from concourse.bass_utils import run_bass_kernel_spmd

N_VIEWS = 40
N_MARKERS = 4
N_TILT = 8
N_OUT_ROWS = 142
PI = float(np.pi)
DEG2RAD = PI / 180.0
HALF_PI = PI / 2.0
CHECK_LIST = np.array([0, 3, 8, 20, 26, 32, 36, 39])
N_CORES = 8

# blob layout ([9, 259], row 0 unless noted):
#   col 0              [1; tilt_angles]             (partitions 0..8)
#   cols 1..41         wmat                         (partitions 0..8)
#   cols 41..57        X'^T rows ordered [3,0,1,2]: cols 41+4j hold the
#                      column of X' pairing with stage segment j
#   col 57             pi/2
#   col 58             0.0
#   cols 59..99        rot_angles (degrees)
#   cols 99..179       [mag_eff | mag_eff]  (view-0 already forced to 1)
#   cols 179..259      off_eff flattened    (view-0 already zeroed)
C_XP = 1 + N_VIEWS           # 41
C_HPI = C_XP + 16            # 57
C_ZERO = C_HPI + 1           # 58
C_ROT = C_ZERO + 1           # 59
C_MAG = C_ROT + N_VIEWS      # 99
C_OFF = C_MAG + 2 * N_VIEWS  # 179
PACK_COLS = C_OFF + 2 * N_VIEWS  # 259

KEEP = np.r_[0:40, 58:160]

AFT = mybir.ActivationFunctionType
F32 = mybir.dt.float32


def _build_wmat() -> np.ndarray:
    views = np.arange(N_VIEWS)
    idx1 = np.searchsorted(CHECK_LIST, views, side="right") - 1
    idx2 = np.minimum(idx1 + 1, len(CHECK_LIST) - 1)
    denom = (CHECK_LIST[idx2] - CHECK_LIST[idx1]).astype(np.float32)
    denom[denom == 0] = 1.0
    frac = (views - CHECK_LIST[idx1]).astype(np.float32) / denom
    w = np.zeros((N_VIEWS, N_TILT), dtype=np.float64)
    c = np.zeros(N_VIEWS, dtype=np.float64)
    for v in range(N_VIEWS):
        if v == 14:
            c[v] = -15.0
        else:
            w[v, idx1[v]] += 1.0 - float(frac[v])
            w[v, idx2[v]] += float(frac[v])
    wmat = np.concatenate([c[None, :], w.T], axis=0) * DEG2RAD
    return np.ascontiguousarray(wmat, dtype=np.float32)


_WMAT = _build_wmat()
_NC_CACHE: list = []


def _chain(insts):
    for a, b in zip(insts, insts[1:]):
        add_dep_helper(b.ins, a.ins, sync=False, reason="pin engine order")


def _legalize_multiwait(nc) -> None:
    """walrus fits one sem-wait per instruction; hoist extras onto
    single-wait EventSemaphore carriers."""
    for fn in nc.m.functions:
        for blk in fn.blocks:
            il = blk.instructions
            i = 0
            while i < len(il):
                inst = il[i]
                si = inst.sync_info
                if si is not None and si.on_wait is not None and len(si.on_wait) > 1:
                    waits = list(si.on_wait)
                    extras, keep = waits[:-1], waits[-1]
                    for j, w in enumerate(extras):
                        ev = mybir.InstEventSemaphore(
                            name=f"{inst.name}_wsplit{j}")
                        ev.engine = inst.engine
                        try:
                            ev.sync_info.on_wait = [w]
                        except Exception:
                            ev.sync_info = mybir.SyncInfo(on_wait=[w],
                                                          on_update=[])
                        il.insert(i, ev)
                        i += 1
                    si.on_wait = [keep]
                i += 1


def _strip_preamble(nc) -> None:
    """Drop the const-AP memsets and the init all-engine barrier (nothing
    uses the const APs; all cross-engine ordering is via tile sems)."""
    il = nc.m.functions[0].blocks[0].instructions
    keep = []
    for inst in il:
        nm = type(inst).__name__
        if nm == "InstMemset" and "const-" in str(inst.outs[0]):
            continue
        if nm in ("InstDrain", "InstEventSemaphore"):
            continue
        keep.append(inst)
    il[:] = keep


def _build_nc(postpasses: bool = True) -> bass.Bass:
    nc = bass.Bass("TRN2", target_bir_lowering=False, debug=False,
                   num_devices=N_CORES)

    V = N_VIEWS
    W2 = 2 * V
    pack_d = nc.dram_tensor("pack", [N_TILT + 1, PACK_COLS], F32,
                            kind="ExternalInput")
    out_d = nc.dram_tensor("out", [4 * V, 2], F32, kind="ExternalOutput")

    with tile.TileContext(nc) as tc:
        with (
            tc.tile_pool(name="sb", bufs=1) as sb,
            tc.tile_pool(name="ps", bufs=1, space="PSUM") as ps,
        ):
            pk = sb.tile([N_TILT + 1, PACK_COLS], F32)
            tilt_ps = ps.tile([1, V], F32)
            abs_t = sb.tile([1, V], F32)
            crsr = sb.tile([1, W2], F32)   # [cos(rot) | sin(rot)]
            ct = sb.tile([1, V], F32)
            st = sb.tile([1, V], F32)
            mcs = sb.tile([1, W2], F32)    # [mag*cos | mag*sin]
            obs = sb.tile([1, 1], F32)
            stage = sb.tile([1, 3 * W2], F32)
            uv_ps = ps.tile([N_MARKERS, W2], F32)
            out_sb = sb.tile([N_MARKERS, W2], F32)

            halfpi_ap = pk[0:1, C_HPI:C_HPI + 1]
            zero_ap = pk[0:1, C_ZERO:C_ZERO + 1]
            rot_ap = pk[0:1, C_ROT:C_ROT + V]
            magmag_ap = pk[0:1, C_MAG:C_MAG + W2]
            off_ap = pk[0:1, C_OFF:C_OFF + W2]

            # ---- single input DMA --------------------------------------
            d_in = nc.sync.dma_start(pk[:, :], pack_d.ap())

            # ---- PE: tilt radians --------------------------------------
            mm1 = nc.tensor.matmul(tilt_ps[:, :], pk[:, 0:1], pk[:, 1:1 + V])

            # ---- ACT: sr -> abs -> cr -> ct -> st ----------------------
            # <=1 unobserved producer each: sr absorbs the pack DMA, abs
            # absorbs PE, ct waits its own engine (abs), cr/st free.
            a_sr = nc.scalar.activation(crsr[0:1, V:W2], rot_ap, AFT.Sin,
                                        bias=zero_ap, scale=DEG2RAD)
            a_abs = nc.scalar.activation(abs_t[:, :], tilt_ps[:, :], AFT.Abs,
                                         bias=zero_ap)
            a_cr = nc.scalar.activation(crsr[0:1, 0:V], rot_ap, AFT.Sin,
                                        bias=halfpi_ap, scale=DEG2RAD)
            a_ct = nc.scalar.activation(ct[:, :], abs_t[:, :], AFT.Sin,
                                        bias=halfpi_ap, scale=-1.0)
            a_st = nc.scalar.activation(st[:, :], tilt_ps[:, :], AFT.Sin,
                                        bias=zero_ap)
            _chain([a_sr, a_abs, a_cr, a_ct, a_st])

            # ---- DVE ---------------------------------------------------
            # stage segment j in [j*80,(j+1)*80) is A-component row j,
            # (v,p)-interleaved; the offset row feeds matmul #2 directly
            # from the pack tile.
            s1, s2 = W2, 2 * W2
            # observer: absorbs the ACT fan-in (through ct) so mcs and the
            # products each carry a single fresh wait
            v_ob = nc.vector.tensor_copy(obs[:, :], ct[0:1, 0:1])
            v_mcs = nc.vector.tensor_mul(mcs[:, :], magmag_ap, crsr[:, :])
            mc = mcs[0:1, 0:V]
            ms = mcs[0:1, V:W2]
            v_p1 = nc.vector.tensor_mul(stage[0:1, 0:s1:2], mc, ct[:, :])
            v_p2 = nc.vector.tensor_mul(stage[0:1, 1:s1:2], ms, ct[:, :])
            v_p3 = nc.vector.tensor_scalar_mul(stage[0:1, s1:s2:2], ms, -1.0)
            v_p4 = nc.vector.tensor_copy(stage[0:1, s1 + 1:s2:2], mc)
            v_p5 = nc.vector.tensor_mul(stage[0:1, s2::2], mc, st[:, :])
            v_p6 = nc.vector.tensor_mul(stage[0:1, s2 + 1::2], ms, st[:, :])

            # ---- projection: 4 accumulating rank-1 matmuls -------------
            # uv[m,n] = X'[m,3]*off[n] + sum_j X'[m,j]*stage_seg_j[n]
            rhss = [off_ap, stage[0:1, 0:s1], stage[0:1, s1:s2],
                    stage[0:1, s2:3 * W2]]
            mms = [mm1]
            for j in range(4):
                lhsT = pk[0:1, C_XP + 4 * j:C_XP + 4 * j + 4]
                mms.append(nc.tensor.matmul(uv_ps[:, :], lhsT, rhss[j],
                                            start=(j == 0), stop=(j == 3)))
            _chain(mms)

            v_fin = nc.vector.tensor_copy(out_sb[:, :], uv_ps[:, :])
            _chain([v_ob, v_mcs, v_p1, v_p2, v_p3, v_p4, v_p5, v_p6, v_fin])

            # ---- single output DMA (all 160 rows; host drops 18) -------
            d_out = nc.sync.dma_start(out_d.ap(), out_sb[:, :])
            _chain([d_in, d_out])

    if postpasses:
        _legalize_multiwait(nc)
        _strip_preamble(nc)
    return nc


def _make_in_map(inputs: dict) -> dict:
    tilt = np.ascontiguousarray(inputs["tilt_angles"], dtype=np.float32)
    xyz = np.ascontiguousarray(inputs["xyz"], dtype=np.float32)
    mag_eff = np.ascontiguousarray(inputs["mag"], np.float32).copy()
    mag_eff[0] = 1.0
    off_eff = np.ascontiguousarray(inputs["offset"], np.float32).copy()
    off_eff[0] = 0.0
    xp = np.ones((4, 4), np.float32)
    xp[:, 0:3] = xyz
    pack = np.zeros((N_TILT + 1, PACK_COLS), np.float32)
    pack[0, 0] = 1.0
    pack[1:, 0] = tilt
    pack[:, 1:1 + N_VIEWS] = _WMAT
    pack[0, C_XP:C_HPI] = xp.T[[3, 0, 1, 2]].reshape(-1)
    pack[0, C_HPI] = HALF_PI
    pack[0, C_ROT:C_MAG] = np.ascontiguousarray(inputs["rot_angles"], np.float32)
    pack[0, C_MAG:C_MAG + N_VIEWS] = mag_eff
    pack[0, C_MAG + N_VIEWS:C_OFF] = mag_eff
    pack[0, C_OFF:] = off_eff.reshape(-1)
    return {"pack": pack}


def kernel(**inputs: np.ndarray) -> np.ndarray:
    if not _NC_CACHE:
        _NC_CACHE.append(_build_nc())
    nc = _NC_CACHE[0]

    in_map = _make_in_map(inputs)
    core_ids = list(range(N_CORES))
    res = run_bass_kernel_spmd(nc, [in_map] * N_CORES, core_ids)
    full = np.asarray(res.results[0]["out"], dtype=np.float32)
    return np.ascontiguousarray(full[KEEP])
